# revision 1
# baseline (speedup 1.0000x reference)
"""PairEmbedding Bass kernel for 8 TRN2 NeuronCores.

out[b,i,j,:] = Co[b,j,:] + Cp[b,i,:] + sep(b,i,j) * w_sep
  Co[j] = se_j @ W1 + [0 | pe_j]
  Cp[i] = se_i @ W2 + b_proj + [pe_i | 0]
  sep(i,j) = ln(|aa_i - aa_j| + 1)
where se = emb_table[seq], pe = pos_table[aa_idx], W1 = W_proj[0:144],
W2 = W_proj[144:288], w_sep = W_proj[288].

Sharding: core c -> batch b = c//2, row block i in [128*(c%2), 128*(c%2)+128),
all 256 j. Per-core output (128, 256, 288) f32.
"""

import math
from contextlib import ExitStack

import numpy as np

from concourse import bacc, bass, mybir, tile
from concourse.bass_utils import run_bass_kernel_spmd

dt = mybir.dt
AF = mybir.ActivationFunctionType
ALU = mybir.AluOpType

B = 4
L = 256
D_PAIR = 288
D_HALF = 144
MAX_LEN = 260
VOCAB = 21
IH = 128          # i rows per core
JG = 8            # j's per output DMA group
N_CORES = 8


def _pos_enc_table() -> np.ndarray:
    idx = np.arange(0, D_HALF, 2, dtype=np.float32)
    t = (np.float32(math.log(10000.0)) * idx) / np.float32(D_HALF)
    denom = np.exp(t, dtype=np.float32)
    pos = np.arange(MAX_LEN, dtype=np.float32)[:, None]
    pe = np.zeros((MAX_LEN, D_HALF), dtype=np.float32)
    pe[:, 0::2] = np.sin(pos / denom, dtype=np.float32)
    pe[:, 1::2] = np.cos(pos / denom, dtype=np.float32)
    return pe


def _bcast(ap_src, nparts: int):
    return bass.AP(
        tensor=ap_src.tensor, offset=ap_src.offset, ap=[[0, nparts], *ap_src.ap]
    )


def build(stage: str = "full", repeat: int = 1, variant: str = "") -> bass.Bass:
    nc = bacc.Bacc("TRN2", target_bir_lowering=False)

    seqb_d = nc.dram_tensor("seqb", [L], dt.int32, kind="ExternalInput")
    seqi_d = nc.dram_tensor("seqi", [IH], dt.int32, kind="ExternalInput")
    aab_d = nc.dram_tensor("aab", [L], dt.int32, kind="ExternalInput")
    aai_d = nc.dram_tensor("aai", [IH], dt.int32, kind="ExternalInput")
    emb_d = nc.dram_tensor("emb", [VOCAB, D_HALF], dt.float32, kind="ExternalInput")
    wp_d = nc.dram_tensor("wp", [D_PAIR + 1, D_PAIR], dt.float32, kind="ExternalInput")
    bp_d = nc.dram_tensor("bp", [D_PAIR], dt.float32, kind="ExternalInput")
    out_d = nc.dram_tensor("out", [IH, L, D_PAIR], dt.float32, kind="ExternalOutput")

    # pos-table gather sources, pre-arranged on host: chunk c of <=128 pos
    # rows on partitions, channel slice [0:144] (posL, pe_i) or [144:288]
    # (posR, pe_j), zero elsewhere.
    pos_np = _pos_enc_table()
    posL_np = np.zeros((128, 3 * D_PAIR), dtype=np.float32)
    posR_np = np.zeros((128, 3 * D_PAIR), dtype=np.float32)
    for c in range(3):
        rows = 128 if c < 2 else MAX_LEN - 256
        chunk = pos_np[c * 128 : c * 128 + rows, :]
        posL_np[0:rows, c * D_PAIR : c * D_PAIR + D_HALF] = chunk
        posR_np[0:rows, c * D_PAIR + D_HALF : (c + 1) * D_PAIR] = chunk
    posL_d = nc.inline_tensor(posL_np, "posL_c")
    posR_d = nc.inline_tensor(posR_np, "posR_c")
    iota_np = (
        np.arange(128, dtype=np.float32)[:, None]
        + 128.0 * np.arange(3, dtype=np.float32)[None, :]
    ).astype(np.float32)
    iota_d = nc.inline_tensor(iota_np, "iota")

    with tile.TileContext(nc) as tc, ExitStack() as ctx:
        persist = ctx.enter_context(tc.tile_pool(name="persist", bufs=1))

        # persistent tiles consumed by the j-loop
        flat_t = persist.tile([2, L * D_PAIR], dt.bfloat16, tag="flat")
        ones_t = persist.tile([2, IH], dt.bfloat16, tag="ones")
        cp_t = persist.tile([IH, D_PAIR], dt.float32, tag="cpt")
        wsep_t = persist.tile([IH, D_PAIR], dt.float32, tag="wsep")
        sep_t = persist.tile([IH, L], dt.float32, tag="sept")

        nc.vector.memset(ones_t, 1.0)

        with ExitStack() as pre:
            scr = pre.enter_context(tc.tile_pool(name="scr", bufs=1))
            psc = pre.enter_context(tc.tile_pool(name="psc", bufs=1, space="PSUM"))

            # ---- input loads ----
            iota_t = scr.tile([128, 3], dt.float32, tag="iota")
            nc.sync.dma_start(iota_t, iota_d[:, :])

            emb_t = scr.tile([VOCAB, D_HALF], dt.float32, tag="emb")
            nc.sync.dma_start(emb_t, emb_d[:, :])

            w1a = scr.tile([128, D_PAIR], dt.float32, tag="w1a")
            nc.sync.dma_start(w1a, wp_d[0:128, :])
            w1b = scr.tile([16, D_PAIR], dt.float32, tag="w1b")
            nc.sync.dma_start(w1b, wp_d[128:144, :])
            w2a = scr.tile([128, D_PAIR], dt.float32, tag="w2a")
            nc.sync.dma_start(w2a, wp_d[144:272, :])
            w2b = scr.tile([16, D_PAIR], dt.float32, tag="w2b")
            nc.sync.dma_start(w2b, wp_d[272:288, :])
            nc.sync.dma_start(wsep_t, _bcast(wp_d[288:289, :], 128))

            bp_t = scr.tile([1, D_PAIR], dt.float32, tag="bp")
            nc.sync.dma_start(bp_t, bp_d[:])

            aaB_i = scr.tile([128, L], dt.int32, tag="aaBi")
            nc.sync.dma_start(aaB_i, _bcast(aab_d[:], 128))
            seqB_i = scr.tile([VOCAB, L], dt.int32, tag="seqBi")
            nc.sync.dma_start(seqB_i, _bcast(seqb_d[:], VOCAB))
            seqI_i = scr.tile([VOCAB, IH], dt.int32, tag="seqIi")
            nc.sync.dma_start(seqI_i, _bcast(seqi_d[:], VOCAB))
            aaIB_i = scr.tile([128, IH], dt.int32, tag="aaIBi")
            nc.sync.dma_start(aaIB_i, _bcast(aai_d[:], 128))
            aaCol_i = scr.tile([IH, 1], dt.int32, tag="aaColi")
            nc.sync.dma_start(aaCol_i, aai_d[:])

            posL = scr.tile([128, 3 * D_PAIR], dt.float32, tag="posL")
            nc.sync.dma_start(posL, posL_d[:, :])
            posR = scr.tile([128, 3 * D_PAIR], dt.float32, tag="posR")
            nc.sync.dma_start(posR, posR_d[:, :])

            # ---- int -> f32 casts ----
            aaB_f = scr.tile([128, L], dt.float32, tag="aaBf")
            nc.vector.tensor_copy(aaB_f, aaB_i)
            seqB_f = scr.tile([VOCAB, L], dt.float32, tag="seqBf")
            nc.vector.tensor_copy(seqB_f, seqB_i)
            seqI_f = scr.tile([VOCAB, IH], dt.float32, tag="seqIf")
            nc.vector.tensor_copy(seqI_f, seqI_i)
            aaIB_f = scr.tile([128, IH], dt.float32, tag="aaIBf")
            nc.vector.tensor_copy(aaIB_f, aaIB_i)
            aaCol_f = scr.tile([IH, 1], dt.float32, tag="aaColf")
            nc.vector.tensor_copy(aaCol_f, aaCol_i)

            # ---- one-hots ----
            ohSeq = scr.tile([VOCAB, L], dt.float32, tag="ohSeq")
            nc.vector.tensor_scalar(
                ohSeq, seqB_f, iota_t[0:VOCAB, 0:1], None, ALU.is_equal
            )
            ohSeqI = scr.tile([VOCAB, IH], dt.float32, tag="ohSeqI")
            nc.vector.tensor_scalar(
                ohSeqI, seqI_f, iota_t[0:VOCAB, 0:1], None, ALU.is_equal
            )
            ohP = []
            ohPi = []
            for c in range(3):
                t = scr.tile([128, L], dt.float32, tag=f"ohP{c}", name=f"ohP{c}")
                nc.vector.tensor_scalar(t, aaB_f, iota_t[:, c : c + 1], None, ALU.is_equal)
                ohP.append(t)
                ti = scr.tile([128, IH], dt.float32, tag=f"ohPi{c}", name=f"ohPi{c}")
                nc.vector.tensor_scalar(
                    ti, aaIB_f, iota_t[:, c : c + 1], None, ALU.is_equal
                )
                ohPi.append(ti)

            # ---- seT = emb^T gathered by seq: (144, L) split 128+16 rows ----
            seT_a_ps = psc.tile([128, L], dt.float32, tag="seTaP")
            nc.tensor.matmul(seT_a_ps, emb_t[:, 0:128], ohSeq, start=True, stop=True)
            seT_b_ps = psc.tile([16, L], dt.float32, tag="seTbP")
            nc.tensor.matmul(
                seT_b_ps, emb_t[:, 128:D_HALF], ohSeq, start=True, stop=True
            )
            seT_a = scr.tile([128, L], dt.float32, tag="seTa")
            nc.vector.tensor_copy(seT_a, seT_a_ps)
            seT_b = scr.tile([16, L], dt.float32, tag="seTb")
            nc.vector.tensor_copy(seT_b, seT_b_ps)

            seTi_a_ps = psc.tile([128, IH], dt.float32, tag="seTiaP")
            nc.tensor.matmul(
                seTi_a_ps, emb_t[:, 0:128], ohSeqI, start=True, stop=True
            )
            seTi_b_ps = psc.tile([16, IH], dt.float32, tag="seTibP")
            nc.tensor.matmul(
                seTi_b_ps, emb_t[:, 128:D_HALF], ohSeqI, start=True, stop=True
            )
            seTi_a = scr.tile([128, IH], dt.float32, tag="seTia")
            nc.vector.tensor_copy(seTi_a, seTi_a_ps)
            seTi_b = scr.tile([16, IH], dt.float32, tag="seTib")
            nc.vector.tensor_copy(seTi_b, seTi_b_ps)

            # ---- Co halves -> bf16 hi/lo -> flat layout on partitions 0/32 ----
            for h in range(2):
                co_ps = psc.tile(
                    [128, D_PAIR], dt.float32, tag=f"co{h}", name=f"co{h}"
                )
                sl = slice(h * 128, (h + 1) * 128)
                nc.tensor.matmul(co_ps, seT_a[:, sl], w1a, start=True, stop=False)
                nc.tensor.matmul(co_ps, seT_b[:, sl], w1b, start=False, stop=False)
                for c in range(3):
                    nc.tensor.matmul(
                        co_ps,
                        ohP[c][:, sl],
                        posR[:, c * D_PAIR : (c + 1) * D_PAIR],
                        start=False,
                        stop=(c == 2),
                    )
                co_hi = scr.tile(
                    [128, D_PAIR], dt.bfloat16, tag=f"cohi{h}", name=f"cohi{h}"
                )
                nc.vector.tensor_copy(co_hi, co_ps)
                co_lo = scr.tile(
                    [128, D_PAIR], dt.bfloat16, tag=f"colo{h}", name=f"colo{h}"
                )
                nc.vector.tensor_sub(co_lo, co_ps, co_hi)
                dst = slice(h * 128 * D_PAIR, (h * 128 + 128) * D_PAIR)
                nc.sync.dma_start(flat_t[0:1, dst], co_hi)
                nc.sync.dma_start(flat_t[1:2, dst], co_lo)

            # ---- Cp = se_i @ W2 + b_proj + [pe_i | 0] ----
            ones_f = scr.tile([1, IH], dt.float32, tag="onesf")
            nc.vector.memset(ones_f, 1.0)
            cp_ps = psc.tile([128, D_PAIR], dt.float32, tag="cpP")
            nc.tensor.matmul(cp_ps, seTi_a, w2a, start=True, stop=False)
            nc.tensor.matmul(cp_ps, seTi_b, w2b, start=False, stop=False)
            for c in range(3):
                nc.tensor.matmul(
                    cp_ps,
                    ohPi[c],
                    posL[:, c * D_PAIR : (c + 1) * D_PAIR],
                    start=False,
                    stop=False,
                )
            nc.tensor.matmul(cp_ps, ones_f, bp_t, start=False, stop=True)
            nc.vector.tensor_copy(cp_t, cp_ps)

            # ---- sep = ln(|aa_j - aa_i| + 1) ----
            dist_t = scr.tile([IH, L], dt.float32, tag="dist")
            nc.vector.tensor_scalar(dist_t, aaB_f, aaCol_f, None, ALU.subtract)
            abs_t = scr.tile([IH, L], dt.float32, tag="abs")
            nc.scalar.activation(abs_t, dist_t, AF.Abs)
            nc.scalar.activation(sep_t, abs_t, AF.Ln, bias=1.0)

        if stage == "setup":
            # dump a few persistent tiles into out rows and stop
            dbg = ctx.enter_context(tc.tile_pool(name="dbg", bufs=1))
            dbf = dbg.tile([IH, D_PAIR], dt.float32, tag="dbf")
            nc.vector.tensor_copy(dbf, cp_t)
            nc.sync.dma_start(out_d[:, 0:1, :], dbf)
            nc.vector.tensor_copy(dbf, wsep_t)
            nc.sync.dma_start(out_d[:, 1:2, :], dbf)
            return nc

        # ---- j loop ----
        psj = ctx.enter_context(tc.tile_pool(name="psj", bufs=8, space="PSUM"))
        obp = ctx.enter_context(tc.tile_pool(name="obp", bufs=2))
        ngroups = int(stage[5:]) if stage.startswith("jloop") else L // JG
        if variant == "dmaonly":
            obs = []
            for k in range(2):
                t = obp.tile([IH, JG * D_PAIR], dt.float32, tag="ob", name="ob")
                nc.vector.memset(t, 0.5)
                obs.append(t)
            for g in range(ngroups * repeat):
                g = g % ngroups
                eng = nc.sync if g % 2 == 0 else nc.scalar
                eng.dma_start(out_d[:, g * JG : (g + 1) * JG, :], obs[g % 2])
            return nc
        for g in range(ngroups * repeat):
            g = g % ngroups
            ob = obp.tile([IH, JG * D_PAIR], dt.float32, tag="ob", name="ob")
            for jj in range(JG):
                j = g * JG + jj
                ps = psj.tile([IH, D_PAIR], dt.float32, tag="ps", name="ps")
                nc.tensor.matmul(
                    ps,
                    ones_t[0:2, :],
                    flat_t[0:2, j * D_PAIR : (j + 1) * D_PAIR],
                    start=True,
                    stop=True,
                )
                osl = ob[:, jj * D_PAIR : (jj + 1) * D_PAIR]
                if variant == "nostt":
                    nc.vector.tensor_copy(osl, ps)
                elif variant == "sttsb":
                    nc.vector.scalar_tensor_tensor(
                        osl, wsep_t, sep_t[:, j : j + 1], cp_t, ALU.mult, ALU.add
                    )
                else:
                    nc.vector.scalar_tensor_tensor(
                        osl, wsep_t, sep_t[:, j : j + 1], ps, ALU.mult, ALU.add
                    )
                if variant not in ("nopool", "nostt", "sttsb"):
                    nc.gpsimd.tensor_add(osl, osl, cp_t)
            if variant != "nodma":
                eng = nc.sync if g % 2 == 0 else nc.scalar
                eng.dma_start(out_d[:, g * JG : (g + 1) * JG, :], ob)

    return nc


_NC_CACHE = []


def make_in_maps(seq, aa_idx, emb_table, W_proj, b_proj):
    seq = np.asarray(seq, dtype=np.int32)
    aa_idx = np.asarray(aa_idx, dtype=np.int32)
    emb_table = np.ascontiguousarray(np.asarray(emb_table, dtype=np.float32))
    W_proj = np.ascontiguousarray(np.asarray(W_proj, dtype=np.float32))
    b_proj = np.ascontiguousarray(np.asarray(b_proj, dtype=np.float32))
    in_maps = []
    for c in range(N_CORES):
        b, ih = c // 2, c % 2
        in_maps.append(
            {
                "seqb": np.ascontiguousarray(seq[b]),
                "seqi": np.ascontiguousarray(seq[b, ih * IH : (ih + 1) * IH]),
                "aab": np.ascontiguousarray(aa_idx[b]),
                "aai": np.ascontiguousarray(aa_idx[b, ih * IH : (ih + 1) * IH]),
                "emb": emb_table,
                "wp": W_proj,
                "bp": b_proj,
            }
        )
    return in_maps


def gather_out(results) -> np.ndarray:
    out = np.empty((B, L, L, D_PAIR), dtype=np.float32)
    for c in range(N_CORES):
        b, ih = c // 2, c % 2
        out[b, ih * IH : (ih + 1) * IH] = np.asarray(results[c]["out"])
    return out


def kernel(seq, aa_idx, emb_table, W_proj, b_proj) -> np.ndarray:
    if not _NC_CACHE:
        nc = build()
        nc.finalize()
        _NC_CACHE.append(nc)
    nc = _NC_CACHE[0]
    in_maps = make_in_maps(seq, aa_idx, emb_table, W_proj, b_proj)
    res = run_bass_kernel_spmd(nc, in_maps, core_ids=list(range(N_CORES)))
    return gather_out(res.results)



# revision 27
# speedup vs baseline: 2.0358x; 2.0358x over previous
"""PairEmbedding Bass kernel for 8 TRN2 NeuronCores.

out[b,i,j,:] = Co[b,j,:] + Cp[b,i,:] + sep(b,i,j) * w_sep
  Co[j] = se_j @ W1 + [0 | pe_j]
  Cp[i] = se_i @ W2 + b_proj + [pe_i | 0]
  sep(i,j) = ln(|aa_i - aa_j| + 1)
where se = emb_table[seq], pe = pos_table[aa_idx], W1 = W_proj[0:144],
W2 = W_proj[144:288], w_sep = W_proj[288].

Sharding: core c -> batch b = c//2, row block i in [128*(c%2), 128*(c%2)+128),
all 256 j. Per-core output (128, 256, 288) stored bf16, host upcasts to f32
(bf16 rounding ~0.4% << the 2e-2 gate).

Per-j work (j-loop), spread across all five engines:
  TensorE: ps(f32 psum) = ones x co_bf[j] (K=1) then += sepT_bf[j] x wsep (K=1)
  drain:   either DVE   ob_bf16 = ps + cp_bf            (fraction DVE_NUM/DVE_DEN)
           or     ACT   ob_bf16 = Identity(ps); GPSIMD ob += cp_bf
  DMA:     groups of JG j's -> HBM, alternating sync/scalar HWDGE queues.
Matmul operands are indexed along the free dim from base partitions {0, 64}
(HW requires base partition in {0,32,64}).
"""

import math
from contextlib import ExitStack

import numpy as np

from concourse import bacc, bass, mybir, tile
from concourse.bass_utils import run_bass_kernel_spmd

dt = mybir.dt
AF = mybir.ActivationFunctionType
ALU = mybir.AluOpType

B = 4
L = 256
D_PAIR = 288
D_HALF = 144
MAX_LEN = 260
VOCAB = 21
IH = 128          # i rows per core
JG = 32           # j's per output DMA group
N_CORES = 8


def _pos_enc_table() -> np.ndarray:
    idx = np.arange(0, D_HALF, 2, dtype=np.float32)
    t = (np.float32(math.log(10000.0)) * idx) / np.float32(D_HALF)
    denom = np.exp(t, dtype=np.float32)
    pos = np.arange(MAX_LEN, dtype=np.float32)[:, None]
    pe = np.zeros((MAX_LEN, D_HALF), dtype=np.float32)
    pe[:, 0::2] = np.sin(pos / denom, dtype=np.float32)
    pe[:, 1::2] = np.cos(pos / denom, dtype=np.float32)
    return pe


def _bcast(ap_src, nparts: int):
    return bass.AP(
        tensor=ap_src.tensor, offset=ap_src.offset, ap=[[0, nparts], *ap_src.ap]
    )


def _pattern_seq(pat: str):
    """'P:64,Q:64,V:96,W:32' -> interleaved length-256 list of pattern codes."""
    counts = {}
    for part in pat.split(","):
        k, v = part.split(":")
        counts[k.strip()] = int(v)
    assert sum(counts.values()) == L
    seq = []
    used = {k: 0 for k in counts}
    for i in range(1, L + 1):
        k = max(counts, key=lambda c: counts[c] * i / L - used[c])
        seq.append(k)
        used[k] += 1
    return seq


def build(
    stage: str = "full",
    repeat: int = 1,
    variant: str = "",
    jg: int = JG,
    pat: str = "P:64,Q:64,V:96,W:32",
) -> bass.Bass:
    nc = bacc.Bacc("TRN2", target_bir_lowering=False)

    seqb_d = nc.dram_tensor("seqb", [L], dt.int32, kind="ExternalInput")
    seqi_d = nc.dram_tensor("seqi", [IH], dt.int32, kind="ExternalInput")
    aab_d = nc.dram_tensor("aab", [L], dt.int32, kind="ExternalInput")
    aai_d = nc.dram_tensor("aai", [IH], dt.int32, kind="ExternalInput")
    emb_d = nc.dram_tensor("emb", [VOCAB, D_HALF], dt.float32, kind="ExternalInput")
    wp_d = nc.dram_tensor("wp", [D_PAIR + 1, D_PAIR], dt.float32, kind="ExternalInput")
    bp_d = nc.dram_tensor("bp", [D_PAIR], dt.float32, kind="ExternalInput")
    out_d = nc.dram_tensor("out", [IH, L, D_PAIR], dt.bfloat16, kind="ExternalOutput")

    # pos-table gather sources, pre-arranged on host: chunk c of <=128 pos
    # rows on partitions, channel slice [0:144] (posL, pe_i) or [144:288]
    # (posR, pe_j), zero elsewhere.
    pos_np = _pos_enc_table()
    posL_np = np.zeros((128, 3 * D_PAIR), dtype=np.float32)
    posR_np = np.zeros((128, 3 * D_PAIR), dtype=np.float32)
    for c in range(3):
        rows = 128 if c < 2 else MAX_LEN - 256
        chunk = pos_np[c * 128 : c * 128 + rows, :]
        posL_np[0:rows, c * D_PAIR : c * D_PAIR + D_HALF] = chunk
        posR_np[0:rows, c * D_PAIR + D_HALF : (c + 1) * D_PAIR] = chunk
    posL_d = nc.inline_tensor(posL_np, "posL_c")
    posR_d = nc.inline_tensor(posR_np, "posR_c")
    iota_np = (
        np.arange(128, dtype=np.float32)[:, None]
        + 128.0 * np.arange(3, dtype=np.float32)[None, :]
    ).astype(np.float32)
    iota_d = nc.inline_tensor(iota_np, "iota")

    with tile.TileContext(nc) as tc, ExitStack() as ctx:
        persist = ctx.enter_context(tc.tile_pool(name="persist", bufs=1))

        # persistent tiles consumed by the j-loop. Matmul tables keep rows at
        # base partitions 0 (j<128) and 64 (j>=128).
        co_t = persist.tile([65, 128 * D_PAIR], dt.bfloat16, tag="co")
        # sepT rows for the per-j sep matmul: sepT[64h, (j%128)*128+i] = sep[i, j]
        sepT_t = persist.tile([65, 128 * IH], dt.bfloat16, tag="sepT")
        ws_t = persist.tile([65, D_PAIR], dt.bfloat16, tag="ws")
        ones_t = persist.tile([65, IH], dt.bfloat16, tag="ones")
        cp_t = persist.tile([IH, D_PAIR], dt.bfloat16, tag="cpt")
        wsepB_t = persist.tile([IH, D_PAIR], dt.bfloat16, tag="wsepB")
        sep_t = persist.tile([IH, L], dt.float32, tag="sept")

        nc.vector.memset(ones_t, 1.0)

        _qs = [nc.sync, nc.scalar]
        _qi = [0]

        def _q():
            _qi[0] ^= 1
            return _qs[_qi[0]]

        with ExitStack() as pre:
            scr = pre.enter_context(tc.tile_pool(name="scr", bufs=1))
            psc = pre.enter_context(tc.tile_pool(name="psc", bufs=1, space="PSUM"))

            # ---- input loads ----
            iota_t = scr.tile([128, 3], dt.float32, tag="iota")
            _q().dma_start(iota_t, iota_d[:, :])

            emb_t = scr.tile([VOCAB, D_HALF], dt.float32, tag="emb")
            _q().dma_start(emb_t, emb_d[:, :])

            w1a = scr.tile([128, D_PAIR], dt.float32, tag="w1a")
            _q().dma_start(w1a, wp_d[0:128, :])
            w1b = scr.tile([16, D_PAIR], dt.float32, tag="w1b")
            _q().dma_start(w1b, wp_d[128:144, :])
            w2a = scr.tile([128, D_PAIR], dt.float32, tag="w2a")
            _q().dma_start(w2a, wp_d[144:272, :])
            w2b = scr.tile([16, D_PAIR], dt.float32, tag="w2b")
            _q().dma_start(w2b, wp_d[272:288, :])

            wsep_f = scr.tile([1, D_PAIR], dt.float32, tag="wsepf")
            _q().dma_start(wsep_f, wp_d[288:289, :])

            bp_t = scr.tile([1, D_PAIR], dt.float32, tag="bp")
            _q().dma_start(bp_t, bp_d[:])

            aab_col = scr.tile([128, 2], dt.int32, tag="aabcol")
            _q().dma_start(
                aab_col,
                bass.AP(tensor=aab_d, offset=0, ap=[[1, 128], [128, 2]]),
            )
            aaB_i = scr.tile([128, L], dt.int32, tag="aaBi")
            _q().dma_start(aaB_i, _bcast(aab_d[:], 128))
            seqB_i = scr.tile([VOCAB, L], dt.int32, tag="seqBi")
            _q().dma_start(seqB_i, _bcast(seqb_d[:], VOCAB))
            seqI_i = scr.tile([VOCAB, IH], dt.int32, tag="seqIi")
            _q().dma_start(seqI_i, _bcast(seqi_d[:], VOCAB))
            aaIB_i = scr.tile([128, IH], dt.int32, tag="aaIBi")
            _q().dma_start(aaIB_i, _bcast(aai_d[:], 128))

            posL = scr.tile([128, 3 * D_PAIR], dt.float32, tag="posL")
            _q().dma_start(posL, posL_d[:, :])
            posR = scr.tile([128, 3 * D_PAIR], dt.float32, tag="posR")
            _q().dma_start(posR, posR_d[:, :])

            # ---- int -> f32 casts ----
            aab_colf = scr.tile([128, 2], dt.float32, tag="aabcolf")
            nc.vector.tensor_copy(aab_colf, aab_col)
            aaB_f = scr.tile([128, L], dt.float32, tag="aaBf")
            nc.vector.tensor_copy(aaB_f, aaB_i)
            seqB_f = scr.tile([VOCAB, L], dt.float32, tag="seqBf")
            nc.vector.tensor_copy(seqB_f, seqB_i)
            seqI_f = scr.tile([VOCAB, IH], dt.float32, tag="seqIf")
            nc.vector.tensor_copy(seqI_f, seqI_i)
            aaIB_f = scr.tile([128, IH], dt.float32, tag="aaIBf")
            nc.vector.tensor_copy(aaIB_f, aaIB_i)

            # ---- one-hots ----
            ohSeq = scr.tile([VOCAB, L], dt.float32, tag="ohSeq")
            nc.vector.tensor_scalar(
                ohSeq, seqB_f, iota_t[0:VOCAB, 0:1], None, ALU.is_equal
            )
            ohSeqI = scr.tile([VOCAB, IH], dt.float32, tag="ohSeqI")
            nc.vector.tensor_scalar(
                ohSeqI, seqI_f, iota_t[0:VOCAB, 0:1], None, ALU.is_equal
            )
            ohP = []
            ohPi = []
            for c in range(3):
                t = scr.tile([128, L], dt.float32, tag=f"ohP{c}", name=f"ohP{c}")
                nc.vector.tensor_scalar(t, aaB_f, iota_t[:, c : c + 1], None, ALU.is_equal)
                ohP.append(t)
                ti = scr.tile([128, IH], dt.float32, tag=f"ohPi{c}", name=f"ohPi{c}")
                nc.vector.tensor_scalar(
                    ti, aaIB_f, iota_t[:, c : c + 1], None, ALU.is_equal
                )
                ohPi.append(ti)

            # ---- seT = emb^T gathered by seq: (144, L) split 128+16 rows ----
            seT_a_ps = psc.tile([128, L], dt.float32, tag="seTaP")
            nc.tensor.matmul(seT_a_ps, emb_t[:, 0:128], ohSeq, start=True, stop=True)
            seT_b_ps = psc.tile([16, L], dt.float32, tag="seTbP")
            nc.tensor.matmul(
                seT_b_ps, emb_t[:, 128:D_HALF], ohSeq, start=True, stop=True
            )
            seT_a = scr.tile([128, L], dt.float32, tag="seTa")
            nc.vector.tensor_copy(seT_a, seT_a_ps)
            seT_b = scr.tile([16, L], dt.float32, tag="seTb")
            nc.vector.tensor_copy(seT_b, seT_b_ps)

            seTi_a_ps = psc.tile([128, IH], dt.float32, tag="seTiaP")
            nc.tensor.matmul(
                seTi_a_ps, emb_t[:, 0:128], ohSeqI, start=True, stop=True
            )
            seTi_b_ps = psc.tile([16, IH], dt.float32, tag="seTibP")
            nc.tensor.matmul(
                seTi_b_ps, emb_t[:, 128:D_HALF], ohSeqI, start=True, stop=True
            )
            seTi_a = scr.tile([128, IH], dt.float32, tag="seTia")
            nc.vector.tensor_copy(seTi_a, seTi_a_ps)
            seTi_b = scr.tile([16, IH], dt.float32, tag="seTib")
            nc.vector.tensor_copy(seTi_b, seTi_b_ps)

            # ---- Co halves -> bf16 -> flat rows at partitions 0 / 64 ----
            for h in range(2):
                co_ps = psc.tile(
                    [128, D_PAIR], dt.float32, tag=f"co{h}", name=f"co{h}"
                )
                sl = slice(h * 128, (h + 1) * 128)
                nc.tensor.matmul(co_ps, seT_a[:, sl], w1a, start=True, stop=False)
                nc.tensor.matmul(co_ps, seT_b[:, sl], w1b, start=False, stop=False)
                for c in range(3):
                    nc.tensor.matmul(
                        co_ps,
                        ohP[c][:, sl],
                        posR[:, c * D_PAIR : (c + 1) * D_PAIR],
                        start=False,
                        stop=(c == 2),
                    )
                co_bf = scr.tile(
                    [128, D_PAIR], dt.bfloat16, tag=f"cobf{h}", name=f"cobf{h}"
                )
                nc.vector.tensor_copy(co_bf, co_ps)
                _q().dma_start(co_t[64 * h : 64 * h + 1, :], co_bf)

            # ---- Cp = se_i @ W2 + b_proj + [pe_i | 0], bf16 ----
            ones_f = scr.tile([1, IH], dt.float32, tag="onesf")
            nc.vector.memset(ones_f, 1.0)
            cp_ps = psc.tile([128, D_PAIR], dt.float32, tag="cpP")
            nc.tensor.matmul(cp_ps, seTi_a, w2a, start=True, stop=False)
            nc.tensor.matmul(cp_ps, seTi_b, w2b, start=False, stop=False)
            for c in range(3):
                nc.tensor.matmul(
                    cp_ps,
                    ohPi[c],
                    posL[:, c * D_PAIR : (c + 1) * D_PAIR],
                    start=False,
                    stop=False,
                )
            nc.tensor.matmul(cp_ps, ones_f, bp_t, start=False, stop=True)
            nc.vector.tensor_copy(cp_t, cp_ps)

            # ---- sepT_h[jj, i] = ln(|aa_i - aa_j|+1) -> S8 batched rows ----
            for h in range(2):
                dist_t = scr.tile([128, IH], dt.float32, tag=f"dist{h}", name=f"dist{h}")
                nc.vector.tensor_scalar(
                    dist_t, aaIB_f, aab_colf[:, h : h + 1], None, ALU.subtract
                )
                abs_t = scr.tile([128, IH], dt.float32, tag=f"abs{h}", name=f"abs{h}")
                nc.scalar.activation(abs_t, dist_t, AF.Abs)
                sep_f = scr.tile([128, IH], dt.float32, tag=f"sepf{h}", name=f"sepf{h}")
                nc.scalar.activation(sep_f, abs_t, AF.Ln, bias=1.0)
                sep_bf = scr.tile(
                    [128, IH], dt.bfloat16, tag=f"sepbf{h}", name=f"sepbf{h}"
                )
                nc.vector.tensor_copy(sep_bf, sep_f)
                _q().dma_start(sepT_t[64 * h : 64 * h + 1, :], sep_bf)

            # ---- Z selector from wsep ----
            wsep_bf = scr.tile([1, D_PAIR], dt.bfloat16, tag="wsepbf")
            nc.vector.tensor_copy(wsep_bf, wsep_f)
            _q().dma_start(ws_t[0:1, :], wsep_bf)
            _q().dma_start(ws_t[64:65, :], wsep_bf)

            # ---- wsep broadcast rows + sep in i-partition layout (V/W) ----
            wsepF_t = scr.tile([IH, D_PAIR], dt.float32, tag="wsepF")
            _q().dma_start(wsepF_t, _bcast(wp_d[288:289, :], 128))
            nc.vector.tensor_copy(wsepB_t, wsepF_t)
            aaCol_i = scr.tile([IH, 1], dt.int32, tag="aaColi")
            _q().dma_start(aaCol_i, aai_d[:])
            aaCol_f = scr.tile([IH, 1], dt.float32, tag="aaColf")
            nc.vector.tensor_copy(aaCol_f, aaCol_i)
            distI_t = scr.tile([IH, L], dt.float32, tag="distI")
            nc.vector.tensor_scalar(distI_t, aaB_f, aaCol_f, None, ALU.subtract)
            absI_t = scr.tile([IH, L], dt.float32, tag="absI")
            nc.scalar.activation(absI_t, distI_t, AF.Abs)
            nc.scalar.activation(sep_t, absI_t, AF.Ln, bias=1.0)

        if stage == "setup":
            dbg = ctx.enter_context(tc.tile_pool(name="dbg", bufs=1))
            dbf = dbg.tile([IH, D_PAIR], dt.bfloat16, tag="dbf")
            nc.vector.tensor_copy(dbf, cp_t)
            nc.sync.dma_start(out_d[:, 0:1, :], dbf)
            return nc

        # ---- j loop ----
        psj = ctx.enter_context(tc.tile_pool(name="psj", bufs=8, space="PSUM"))
        obp = ctx.enter_context(tc.tile_pool(name="obp", bufs=2))
        ngroups = int(stage[5:]) if stage.startswith("jloop") else L // jg
        if variant == "dmaonly":
            obs = []
            for k in range(2):
                t = obp.tile([IH, jg * D_PAIR], dt.bfloat16, tag="ob", name="ob")
                nc.vector.memset(t, 0.5)
                obs.append(t)
            for g in range(ngroups * repeat):
                g = g % ngroups
                eng = nc.sync if g % 2 == 0 else nc.scalar
                eng.dma_start(out_d[:, g * jg : (g + 1) * jg, :], obs[g % 2])
            return nc
        vset = set(variant.split("+")) if variant else set()
        seq = _pattern_seq(pat)
        if "allP" in vset:
            seq = ["P"] * L
        SUB = 8
        for g in range(ngroups * repeat):
            g = g % ngroups
            ob = obp.tile([IH, jg * D_PAIR], dt.bfloat16, tag="ob", name="ob")
            for s in range(jg // SUB):
                js = [g * jg + s * SUB + t for t in range(SUB)]
                pss = []
                for j in js:
                    b, jo = 64 * (j // 128), j % 128
                    code = seq[j]
                    ps = psj.tile([IH, D_PAIR], dt.float32, tag="ps", name="ps")
                    pe_sep = code in ("P", "Q", "B") and "mm1" not in vset
                    nc.tensor.matmul(
                        ps,
                        ones_t[b : b + 1, :],
                        co_t[b : b + 1, jo * D_PAIR : (jo + 1) * D_PAIR],
                        start=True,
                        stop=not pe_sep,
                    )
                    pss.append(ps)
                for ps, j in zip(pss, js):
                    code = seq[j]
                    if code in ("P", "Q", "B") and "mm1" not in vset:
                        b, jo = 64 * (j // 128), j % 128
                        nc.tensor.matmul(
                            ps,
                            sepT_t[b : b + 1, jo * IH : (jo + 1) * IH],
                            ws_t[b : b + 1, :],
                            start=False,
                            stop=True,
                        )
                for ps, j in zip(pss, js):
                    jj = j - g * jg
                    osl = ob[:, jj * D_PAIR : (jj + 1) * D_PAIR]
                    code = seq[j]
                    if "nodrain" in vset:
                        continue
                    if "actonly" in vset:
                        nc.scalar.activation(osl, ps, AF.Identity)
                        continue
                    if code == "P":
                        nc.vector.tensor_tensor(osl, ps, cp_t, ALU.add)
                    elif code == "Q":
                        nc.scalar.activation(osl, ps, AF.Identity)
                        nc.gpsimd.tensor_tensor(osl, osl, cp_t, ALU.add)
                    elif code == "B":
                        nc.scalar.activation(osl, ps, AF.Identity)
                        nc.vector.tensor_tensor(osl, osl, cp_t, ALU.add)
                    elif code == "V":
                        nc.vector.scalar_tensor_tensor(
                            osl, wsepB_t, sep_t[:, j : j + 1], ps, ALU.mult, ALU.add
                        )
                        nc.gpsimd.tensor_tensor(osl, osl, cp_t, ALU.add)
                    else:  # W
                        nc.vector.scalar_tensor_tensor(
                            osl, wsepB_t, sep_t[:, j : j + 1], ps, ALU.mult, ALU.add
                        )
                        nc.vector.tensor_tensor(osl, osl, cp_t, ALU.add)
            if "nodma" not in vset and "nodrain" not in vset:
                eng = nc.sync if (g % 2 == 0 or "allsync" in vset) else nc.scalar
                eng.dma_start(out_d[:, g * jg : (g + 1) * jg, :], ob)

    return nc


_NC_CACHE = []


def make_in_maps(seq, aa_idx, emb_table, W_proj, b_proj):
    seq = np.asarray(seq, dtype=np.int32)
    aa_idx = np.asarray(aa_idx, dtype=np.int32)
    emb_table = np.ascontiguousarray(np.asarray(emb_table, dtype=np.float32))
    W_proj = np.ascontiguousarray(np.asarray(W_proj, dtype=np.float32))
    b_proj = np.ascontiguousarray(np.asarray(b_proj, dtype=np.float32))
    in_maps = []
    for c in range(N_CORES):
        b, ih = c // 2, c % 2
        in_maps.append(
            {
                "seqb": np.ascontiguousarray(seq[b]),
                "seqi": np.ascontiguousarray(seq[b, ih * IH : (ih + 1) * IH]),
                "aab": np.ascontiguousarray(aa_idx[b]),
                "aai": np.ascontiguousarray(aa_idx[b, ih * IH : (ih + 1) * IH]),
                "emb": emb_table,
                "wp": W_proj,
                "bp": b_proj,
            }
        )
    return in_maps


def gather_out(results) -> np.ndarray:
    out = np.empty((B, L, L, D_PAIR), dtype=np.float32)
    for c in range(N_CORES):
        b, ih = c // 2, c % 2
        out[b, ih * IH : (ih + 1) * IH] = np.asarray(results[c]["out"]).astype(
            np.float32
        )
    return out


def kernel(seq, aa_idx, emb_table, W_proj, b_proj) -> np.ndarray:
    if not _NC_CACHE:
        nc = build()
        nc.finalize()
        _NC_CACHE.append(nc)
    nc = _NC_CACHE[0]
    in_maps = make_in_maps(seq, aa_idx, emb_table, W_proj, b_proj)
    res = run_bass_kernel_spmd(nc, in_maps, core_ids=list(range(N_CORES)))
    return gather_out(res.results)


# revision 28
# speedup vs baseline: 4.4008x; 2.1618x over previous
"""PairEmbedding Bass kernel for 8 TRN2 NeuronCores.

out[b,i,j,:] = Co[b,j,:] + Cp[b,i,:] + sep(b,i,j) * w_sep
  Co[j] = se_j @ W1 + [0 | pe_j]
  Cp[i] = se_i @ W2 + b_proj + [pe_i | 0]
  sep(i,j) = ln(|aa_i - aa_j| + 1)
where se = emb_table[seq], pe = pos_table[aa_idx], W1 = W_proj[0:144],
W2 = W_proj[144:288], w_sep = W_proj[288].

Sharding: core c -> batch b = c//2, row block i in [128*(c%2), 128*(c%2)+128),
all 256 j. Per-core output (128, 256, 288) stored bf16, host upcasts to f32
(bf16 rounding ~0.4% << the 2e-2 gate).

Per-j work (j-loop), spread across all five engines:
  TensorE: ps(f32 psum) = ones x co_bf[j] (K=1) then += sepT_bf[j] x wsep (K=1)
  drain:   either DVE   ob_bf16 = ps + cp_bf            (fraction DVE_NUM/DVE_DEN)
           or     ACT   ob_bf16 = Identity(ps); GPSIMD ob += cp_bf
  DMA:     groups of JG j's -> HBM, alternating sync/scalar HWDGE queues.
Matmul operands are indexed along the free dim from base partitions {0, 64}
(HW requires base partition in {0,32,64}).
"""

import math
from contextlib import ExitStack

import numpy as np

from concourse import bacc, bass, mybir, tile
from concourse.bass_utils import run_bass_kernel_spmd

dt = mybir.dt
AF = mybir.ActivationFunctionType
ALU = mybir.AluOpType

B = 4
L = 256
D_PAIR = 288
D_HALF = 144
MAX_LEN = 260
VOCAB = 21
IH = 128          # i rows per core
JG = 32           # j's per output DMA group
N_CORES = 8


def _pos_enc_table() -> np.ndarray:
    idx = np.arange(0, D_HALF, 2, dtype=np.float32)
    t = (np.float32(math.log(10000.0)) * idx) / np.float32(D_HALF)
    denom = np.exp(t, dtype=np.float32)
    pos = np.arange(MAX_LEN, dtype=np.float32)[:, None]
    pe = np.zeros((MAX_LEN, D_HALF), dtype=np.float32)
    pe[:, 0::2] = np.sin(pos / denom, dtype=np.float32)
    pe[:, 1::2] = np.cos(pos / denom, dtype=np.float32)
    return pe


def _bcast(ap_src, nparts: int):
    return bass.AP(
        tensor=ap_src.tensor, offset=ap_src.offset, ap=[[0, nparts], *ap_src.ap]
    )


def _pattern_seq(pat: str):
    """'P:64,Q:64,V:96,W:32' -> interleaved length-256 list of pattern codes."""
    counts = {}
    for part in pat.split(","):
        k, v = part.split(":")
        counts[k.strip()] = int(v)
    assert sum(counts.values()) == L
    seq = []
    used = {k: 0 for k in counts}
    for i in range(1, L + 1):
        k = max(counts, key=lambda c: counts[c] * i / L - used[c])
        seq.append(k)
        used[k] += 1
    return seq


def build(
    stage: str = "full",
    repeat: int = 1,
    variant: str = "",
    jg: int = JG,
    pat: str = "P:64,Q:64,V:96,W:32",
    obufs: int = 2,
) -> bass.Bass:
    nc = bacc.Bacc("TRN2", target_bir_lowering=False)

    seqb_d = nc.dram_tensor("seqb", [L], dt.int32, kind="ExternalInput")
    seqi_d = nc.dram_tensor("seqi", [IH], dt.int32, kind="ExternalInput")
    aab_d = nc.dram_tensor("aab", [L], dt.int32, kind="ExternalInput")
    aai_d = nc.dram_tensor("aai", [IH], dt.int32, kind="ExternalInput")
    emb_d = nc.dram_tensor("emb", [VOCAB, D_HALF], dt.float32, kind="ExternalInput")
    wp_d = nc.dram_tensor("wp", [D_PAIR + 1, D_PAIR], dt.float32, kind="ExternalInput")
    bp_d = nc.dram_tensor("bp", [D_PAIR], dt.float32, kind="ExternalInput")
    out_d = nc.dram_tensor("out", [IH, L, D_PAIR], dt.bfloat16, kind="ExternalOutput")

    # pos-table gather sources, pre-arranged on host: chunk c of <=128 pos
    # rows on partitions, channel slice [0:144] (posL, pe_i) or [144:288]
    # (posR, pe_j), zero elsewhere.
    pos_np = _pos_enc_table()
    posL_np = np.zeros((128, 3 * D_PAIR), dtype=np.float32)
    posR_np = np.zeros((128, 3 * D_PAIR), dtype=np.float32)
    for c in range(3):
        rows = 128 if c < 2 else MAX_LEN - 256
        chunk = pos_np[c * 128 : c * 128 + rows, :]
        posL_np[0:rows, c * D_PAIR : c * D_PAIR + D_HALF] = chunk
        posR_np[0:rows, c * D_PAIR + D_HALF : (c + 1) * D_PAIR] = chunk
    posL_d = nc.inline_tensor(posL_np, "posL_c")
    posR_d = nc.inline_tensor(posR_np, "posR_c")
    iota_np = (
        np.arange(128, dtype=np.float32)[:, None]
        + 128.0 * np.arange(3, dtype=np.float32)[None, :]
    ).astype(np.float32)
    iota_d = nc.inline_tensor(iota_np, "iota")

    with tile.TileContext(nc) as tc, ExitStack() as ctx:
        persist = ctx.enter_context(tc.tile_pool(name="persist", bufs=1))

        # persistent tiles consumed by the j-loop. Matmul tables keep rows at
        # base partitions 0 (j<128) and 64 (j>=128).
        co_t = persist.tile([65, 128 * D_PAIR], dt.bfloat16, tag="co")
        # sepT rows for the per-j sep matmul: sepT[64h, (j%128)*128+i] = sep[i, j]
        sepT_t = persist.tile([65, 128 * IH], dt.bfloat16, tag="sepT")
        ws_t = persist.tile([65, D_PAIR], dt.bfloat16, tag="ws")
        ones_t = persist.tile([65, IH], dt.bfloat16, tag="ones")
        cp_t = persist.tile([IH, D_PAIR], dt.bfloat16, tag="cpt")
        wsepB_t = persist.tile([IH, D_PAIR], dt.bfloat16, tag="wsepB")
        sep_t = persist.tile([IH, L], dt.float32, tag="sept")

        nc.vector.memset(ones_t, 1.0)

        _qs = [nc.sync, nc.scalar]
        _qi = [0]

        def _q():
            _qi[0] ^= 1
            return _qs[_qi[0]]

        with ExitStack() as pre:
            scr = pre.enter_context(tc.tile_pool(name="scr", bufs=1))
            psc = pre.enter_context(tc.tile_pool(name="psc", bufs=1, space="PSUM"))

            # ---- input loads ----
            iota_t = scr.tile([128, 3], dt.float32, tag="iota")
            _q().dma_start(iota_t, iota_d[:, :])

            emb_t = scr.tile([VOCAB, D_HALF], dt.float32, tag="emb")
            _q().dma_start(emb_t, emb_d[:, :])

            w1a = scr.tile([128, D_PAIR], dt.float32, tag="w1a")
            _q().dma_start(w1a, wp_d[0:128, :])
            w1b = scr.tile([16, D_PAIR], dt.float32, tag="w1b")
            _q().dma_start(w1b, wp_d[128:144, :])
            w2a = scr.tile([128, D_PAIR], dt.float32, tag="w2a")
            _q().dma_start(w2a, wp_d[144:272, :])
            w2b = scr.tile([16, D_PAIR], dt.float32, tag="w2b")
            _q().dma_start(w2b, wp_d[272:288, :])

            wsep_f = scr.tile([1, D_PAIR], dt.float32, tag="wsepf")
            _q().dma_start(wsep_f, wp_d[288:289, :])

            bp_t = scr.tile([1, D_PAIR], dt.float32, tag="bp")
            _q().dma_start(bp_t, bp_d[:])

            aab_col = scr.tile([128, 2], dt.int32, tag="aabcol")
            _q().dma_start(
                aab_col,
                bass.AP(tensor=aab_d, offset=0, ap=[[1, 128], [128, 2]]),
            )
            aaB_i = scr.tile([128, L], dt.int32, tag="aaBi")
            _q().dma_start(aaB_i, _bcast(aab_d[:], 128))
            seqB_i = scr.tile([VOCAB, L], dt.int32, tag="seqBi")
            _q().dma_start(seqB_i, _bcast(seqb_d[:], VOCAB))
            seqI_i = scr.tile([VOCAB, IH], dt.int32, tag="seqIi")
            _q().dma_start(seqI_i, _bcast(seqi_d[:], VOCAB))
            aaIB_i = scr.tile([128, IH], dt.int32, tag="aaIBi")
            _q().dma_start(aaIB_i, _bcast(aai_d[:], 128))

            posL = scr.tile([128, 3 * D_PAIR], dt.float32, tag="posL")
            _q().dma_start(posL, posL_d[:, :])
            posR = scr.tile([128, 3 * D_PAIR], dt.float32, tag="posR")
            _q().dma_start(posR, posR_d[:, :])

            # ---- int -> f32 casts ----
            aab_colf = scr.tile([128, 2], dt.float32, tag="aabcolf")
            nc.vector.tensor_copy(aab_colf, aab_col)
            aaB_f = scr.tile([128, L], dt.float32, tag="aaBf")
            nc.vector.tensor_copy(aaB_f, aaB_i)
            seqB_f = scr.tile([VOCAB, L], dt.float32, tag="seqBf")
            nc.vector.tensor_copy(seqB_f, seqB_i)
            seqI_f = scr.tile([VOCAB, IH], dt.float32, tag="seqIf")
            nc.vector.tensor_copy(seqI_f, seqI_i)
            aaIB_f = scr.tile([128, IH], dt.float32, tag="aaIBf")
            nc.vector.tensor_copy(aaIB_f, aaIB_i)

            # ---- one-hots ----
            ohSeq = scr.tile([VOCAB, L], dt.float32, tag="ohSeq")
            nc.vector.tensor_scalar(
                ohSeq, seqB_f, iota_t[0:VOCAB, 0:1], None, ALU.is_equal
            )
            ohSeqI = scr.tile([VOCAB, IH], dt.float32, tag="ohSeqI")
            nc.vector.tensor_scalar(
                ohSeqI, seqI_f, iota_t[0:VOCAB, 0:1], None, ALU.is_equal
            )
            ohP = []
            ohPi = []
            for c in range(3):
                t = scr.tile([128, L], dt.float32, tag=f"ohP{c}", name=f"ohP{c}")
                nc.vector.tensor_scalar(t, aaB_f, iota_t[:, c : c + 1], None, ALU.is_equal)
                ohP.append(t)
                ti = scr.tile([128, IH], dt.float32, tag=f"ohPi{c}", name=f"ohPi{c}")
                nc.vector.tensor_scalar(
                    ti, aaIB_f, iota_t[:, c : c + 1], None, ALU.is_equal
                )
                ohPi.append(ti)

            # ---- seT = emb^T gathered by seq: (144, L) split 128+16 rows ----
            seT_a_ps = psc.tile([128, L], dt.float32, tag="seTaP")
            nc.tensor.matmul(seT_a_ps, emb_t[:, 0:128], ohSeq, start=True, stop=True)
            seT_b_ps = psc.tile([16, L], dt.float32, tag="seTbP")
            nc.tensor.matmul(
                seT_b_ps, emb_t[:, 128:D_HALF], ohSeq, start=True, stop=True
            )
            seT_a = scr.tile([128, L], dt.float32, tag="seTa")
            nc.vector.tensor_copy(seT_a, seT_a_ps)
            seT_b = scr.tile([16, L], dt.float32, tag="seTb")
            nc.vector.tensor_copy(seT_b, seT_b_ps)

            seTi_a_ps = psc.tile([128, IH], dt.float32, tag="seTiaP")
            nc.tensor.matmul(
                seTi_a_ps, emb_t[:, 0:128], ohSeqI, start=True, stop=True
            )
            seTi_b_ps = psc.tile([16, IH], dt.float32, tag="seTibP")
            nc.tensor.matmul(
                seTi_b_ps, emb_t[:, 128:D_HALF], ohSeqI, start=True, stop=True
            )
            seTi_a = scr.tile([128, IH], dt.float32, tag="seTia")
            nc.vector.tensor_copy(seTi_a, seTi_a_ps)
            seTi_b = scr.tile([16, IH], dt.float32, tag="seTib")
            nc.vector.tensor_copy(seTi_b, seTi_b_ps)

            # ---- Co halves -> bf16 -> flat rows at partitions 0 / 64 ----
            for h in range(2):
                co_ps = psc.tile(
                    [128, D_PAIR], dt.float32, tag=f"co{h}", name=f"co{h}"
                )
                sl = slice(h * 128, (h + 1) * 128)
                nc.tensor.matmul(co_ps, seT_a[:, sl], w1a, start=True, stop=False)
                nc.tensor.matmul(co_ps, seT_b[:, sl], w1b, start=False, stop=False)
                for c in range(3):
                    nc.tensor.matmul(
                        co_ps,
                        ohP[c][:, sl],
                        posR[:, c * D_PAIR : (c + 1) * D_PAIR],
                        start=False,
                        stop=(c == 2),
                    )
                co_bf = scr.tile(
                    [128, D_PAIR], dt.bfloat16, tag=f"cobf{h}", name=f"cobf{h}"
                )
                nc.vector.tensor_copy(co_bf, co_ps)
                _q().dma_start(co_t[64 * h : 64 * h + 1, :], co_bf)

            # ---- Cp = se_i @ W2 + b_proj + [pe_i | 0], bf16 ----
            ones_f = scr.tile([1, IH], dt.float32, tag="onesf")
            nc.vector.memset(ones_f, 1.0)
            cp_ps = psc.tile([128, D_PAIR], dt.float32, tag="cpP")
            nc.tensor.matmul(cp_ps, seTi_a, w2a, start=True, stop=False)
            nc.tensor.matmul(cp_ps, seTi_b, w2b, start=False, stop=False)
            for c in range(3):
                nc.tensor.matmul(
                    cp_ps,
                    ohPi[c],
                    posL[:, c * D_PAIR : (c + 1) * D_PAIR],
                    start=False,
                    stop=False,
                )
            nc.tensor.matmul(cp_ps, ones_f, bp_t, start=False, stop=True)
            nc.vector.tensor_copy(cp_t, cp_ps)

            # ---- sepT_h[jj, i] = ln(|aa_i - aa_j|+1) -> S8 batched rows ----
            for h in range(2):
                dist_t = scr.tile([128, IH], dt.float32, tag=f"dist{h}", name=f"dist{h}")
                nc.vector.tensor_scalar(
                    dist_t, aaIB_f, aab_colf[:, h : h + 1], None, ALU.subtract
                )
                abs_t = scr.tile([128, IH], dt.float32, tag=f"abs{h}", name=f"abs{h}")
                nc.scalar.activation(abs_t, dist_t, AF.Abs)
                sep_f = scr.tile([128, IH], dt.float32, tag=f"sepf{h}", name=f"sepf{h}")
                nc.scalar.activation(sep_f, abs_t, AF.Ln, bias=1.0)
                sep_bf = scr.tile(
                    [128, IH], dt.bfloat16, tag=f"sepbf{h}", name=f"sepbf{h}"
                )
                nc.vector.tensor_copy(sep_bf, sep_f)
                _q().dma_start(sepT_t[64 * h : 64 * h + 1, :], sep_bf)

            # ---- Z selector from wsep ----
            wsep_bf = scr.tile([1, D_PAIR], dt.bfloat16, tag="wsepbf")
            nc.vector.tensor_copy(wsep_bf, wsep_f)
            _q().dma_start(ws_t[0:1, :], wsep_bf)
            _q().dma_start(ws_t[64:65, :], wsep_bf)

            # ---- wsep broadcast rows + sep in i-partition layout (V/W) ----
            wsepF_t = scr.tile([IH, D_PAIR], dt.float32, tag="wsepF")
            _q().dma_start(wsepF_t, _bcast(wp_d[288:289, :], 128))
            nc.vector.tensor_copy(wsepB_t, wsepF_t)
            aaCol_i = scr.tile([IH, 1], dt.int32, tag="aaColi")
            _q().dma_start(aaCol_i, aai_d[:])
            aaCol_f = scr.tile([IH, 1], dt.float32, tag="aaColf")
            nc.vector.tensor_copy(aaCol_f, aaCol_i)
            distI_t = scr.tile([IH, L], dt.float32, tag="distI")
            nc.vector.tensor_scalar(distI_t, aaB_f, aaCol_f, None, ALU.subtract)
            absI_t = scr.tile([IH, L], dt.float32, tag="absI")
            nc.scalar.activation(absI_t, distI_t, AF.Abs)
            nc.scalar.activation(sep_t, absI_t, AF.Ln, bias=1.0)

        if stage == "setup":
            dbg = ctx.enter_context(tc.tile_pool(name="dbg", bufs=1))
            dbf = dbg.tile([IH, D_PAIR], dt.bfloat16, tag="dbf")
            nc.vector.tensor_copy(dbf, cp_t)
            nc.sync.dma_start(out_d[:, 0:1, :], dbf)
            return nc

        # ---- j loop ----
        psj = ctx.enter_context(tc.tile_pool(name="psj", bufs=8, space="PSUM"))
        obp = ctx.enter_context(tc.tile_pool(name="obp", bufs=obufs))
        ngroups = int(stage[5:]) if stage.startswith("jloop") else L // jg
        if variant == "dmaonly":
            obs = []
            for k in range(2):
                t = obp.tile([IH, jg * D_PAIR], dt.bfloat16, tag="ob", name="ob")
                nc.vector.memset(t, 0.5)
                obs.append(t)
            for g in range(ngroups * repeat):
                g = g % ngroups
                eng = nc.sync if g % 2 == 0 else nc.scalar
                eng.dma_start(out_d[:, g * jg : (g + 1) * jg, :], obs[g % 2])
            return nc
        vset = set(variant.split("+")) if variant else set()
        seq = _pattern_seq(pat)
        if "allP" in vset:
            seq = ["P"] * L
        SUB = 8
        for g in range(ngroups * repeat):
            g = g % ngroups
            ob = obp.tile([IH, jg * D_PAIR], dt.bfloat16, tag="ob", name="ob")
            for s in range(jg // SUB):
                js = [g * jg + s * SUB + t for t in range(SUB)]
                pss = []
                for j in js:
                    b, jo = 64 * (j // 128), j % 128
                    code = seq[j]
                    ps = psj.tile([IH, D_PAIR], dt.float32, tag="ps", name="ps")
                    pe_sep = code in ("P", "Q", "B") and "mm1" not in vset
                    nc.tensor.matmul(
                        ps,
                        ones_t[b : b + 1, :],
                        co_t[b : b + 1, jo * D_PAIR : (jo + 1) * D_PAIR],
                        start=True,
                        stop=not pe_sep,
                    )
                    pss.append(ps)
                for ps, j in zip(pss, js):
                    code = seq[j]
                    if code in ("P", "Q", "B") and "mm1" not in vset:
                        b, jo = 64 * (j // 128), j % 128
                        nc.tensor.matmul(
                            ps,
                            sepT_t[b : b + 1, jo * IH : (jo + 1) * IH],
                            ws_t[b : b + 1, :],
                            start=False,
                            stop=True,
                        )
                for ps, j in zip(pss, js):
                    jj = j - g * jg
                    osl = ob[:, jj * D_PAIR : (jj + 1) * D_PAIR]
                    code = seq[j]
                    if "nodrain" in vset:
                        continue
                    if "actonly" in vset:
                        nc.scalar.activation(osl, ps, AF.Identity)
                        continue
                    if code == "P":
                        nc.vector.tensor_tensor(osl, ps, cp_t, ALU.add)
                    elif code == "Q":
                        nc.scalar.activation(osl, ps, AF.Identity)
                        nc.gpsimd.tensor_tensor(osl, osl, cp_t, ALU.add)
                    elif code == "B":
                        nc.scalar.activation(osl, ps, AF.Identity)
                        nc.vector.tensor_tensor(osl, osl, cp_t, ALU.add)
                    elif code == "V":
                        nc.vector.scalar_tensor_tensor(
                            osl, wsepB_t, sep_t[:, j : j + 1], ps, ALU.mult, ALU.add
                        )
                        nc.gpsimd.tensor_tensor(osl, osl, cp_t, ALU.add)
                    else:  # W
                        nc.vector.scalar_tensor_tensor(
                            osl, wsepB_t, sep_t[:, j : j + 1], ps, ALU.mult, ALU.add
                        )
                        nc.vector.tensor_tensor(osl, osl, cp_t, ALU.add)
            if "nodma" not in vset and "nodrain" not in vset:
                eng = nc.sync if (g % 2 == 0 or "allsync" in vset) else nc.scalar
                eng.dma_start(out_d[:, g * jg : (g + 1) * jg, :], ob)

    return nc


_NC_CACHE = []


def make_in_maps(seq, aa_idx, emb_table, W_proj, b_proj):
    seq = np.asarray(seq, dtype=np.int32)
    aa_idx = np.asarray(aa_idx, dtype=np.int32)
    emb_table = np.ascontiguousarray(np.asarray(emb_table, dtype=np.float32))
    W_proj = np.ascontiguousarray(np.asarray(W_proj, dtype=np.float32))
    b_proj = np.ascontiguousarray(np.asarray(b_proj, dtype=np.float32))
    in_maps = []
    for c in range(N_CORES):
        b, ih = c // 2, c % 2
        in_maps.append(
            {
                "seqb": np.ascontiguousarray(seq[b]),
                "seqi": np.ascontiguousarray(seq[b, ih * IH : (ih + 1) * IH]),
                "aab": np.ascontiguousarray(aa_idx[b]),
                "aai": np.ascontiguousarray(aa_idx[b, ih * IH : (ih + 1) * IH]),
                "emb": emb_table,
                "wp": W_proj,
                "bp": b_proj,
            }
        )
    return in_maps


def gather_out(results) -> np.ndarray:
    out = np.empty((B, L, L, D_PAIR), dtype=np.float32)
    for c in range(N_CORES):
        b, ih = c // 2, c % 2
        out[b, ih * IH : (ih + 1) * IH] = np.asarray(results[c]["out"]).astype(
            np.float32
        )
    return out


def kernel(seq, aa_idx, emb_table, W_proj, b_proj) -> np.ndarray:
    if not _NC_CACHE:
        nc = build()
        nc.finalize()
        _NC_CACHE.append(nc)
    nc = _NC_CACHE[0]
    in_maps = make_in_maps(seq, aa_idx, emb_table, W_proj, b_proj)
    res = run_bass_kernel_spmd(nc, in_maps, core_ids=list(range(N_CORES)))
    return gather_out(res.results)
